# revision 17
# baseline (speedup 1.0000x reference)
"""Trainium2 Bass kernel for nn_MultiHeadCrossAttention (B=32, Nc=2048, H=8, topk=12).

kernel(**inputs) takes FULL inputs, returns FULL output [32, 1, 128].
Batch is sharded 4-per-core across 8 NeuronCores (data parallel, no collectives).

Per-batch device algorithm (rows=(h,q) 128 wide, j = e*2048+nc in [0,16384)):
  comp_T fp16 (single term: logit error ~1e-4 << q10 weight quantum 1/128)
  A_e[c,row]  = WkT_e.T @ Qbd -> fp16
  S_e[row,nc] = Ah.T@Ch   (single fp16 matmul per chunk)
  VT_e[hd,nc] = Wv_e.T @ Ch -> VT [128,16384] bf16
  per-chunk(1024) top8 (DVE max8) -> cand [128,128]
  per-chunk max_index -> local indices
  exact global top-12 marking via max8/match_replace rounds on cand
  pack (global_idx*1024 + quantized_value), extract winners via max8
  weights = exp(value)/sum   (selection exact; weight quantization ~0.4%)
  G = ap_gather(VT pairs, winner idx/2; 16-partition cores align with heads)
  PV^T[(h,d),q] = sum w*G  (headrep matmul broadcasts weights, parity split)
  out = (PV flat @ WjwP) + x;  out = out @ Wp + bp
"""

import sys
import numpy as np

for p in ("/opt/trn_rl_repo",):
    if p not in sys.path:
        sys.path.insert(0, p)

import ml_dtypes

B, CORES, BPC = 32, 8, 4
H, HD, NQ, TK, C, NC = 8, 16, 16, 12, 128, 2048
NJ = 8 * NC            # 16384
CHUNK = 2048
NCH = NJ // CHUNK      # 8
NCAND = NCH * 8        # 64
NEG = -1e30
MAGIC = 12582912.0     # 2**23 + 2**22: add/sub rounds fp32 to nearest int

_prog_cache = {}


def _build_program():
    import concourse.bass as bass
    import concourse.mybir as mybir
    import concourse.tile as tile
    from concourse import bacc
    from concourse import library_config

    dt = mybir.dt
    Alu = mybir.AluOpType
    f32, f16, bf16 = dt.float32, dt.float16, dt.bfloat16
    nc = bacc.Bacc("TRN2", target_bir_lowering=False)

    comphT_d = nc.dram_tensor("comphT", [BPC, C, NC], f16, kind="ExternalInput")
    complT_d = nc.dram_tensor("complT", [BPC, C, NC], f16, kind="ExternalInput")
    xT_d = nc.dram_tensor("xT", [C, BPC], f32, kind="ExternalInput")
    wq_d = nc.dram_tensor("wq", [C, 2048], f32, kind="ExternalInput")
    wkT_d = nc.dram_tensor("wkT", [C, 8 * C], f32, kind="ExternalInput")
    wv_d = nc.dram_tensor("wv", [C, 8 * C], f16, kind="ExternalInput")
    wjwp_d = nc.dram_tensor("wjwp", [C, NQ * C], f32, kind="ExternalInput")
    wp_d = nc.dram_tensor("wp", [C, C], f32, kind="ExternalInput")
    bp4_d = nc.dram_tensor("bp4", [BPC, C], f32, kind="ExternalInput")
    hrep_d = nc.dram_tensor("hrep", [C, C], f32, kind="ExternalInput")
    choff_d = nc.dram_tensor("choff", [C, NCAND], f32, kind="ExternalInput")
    me_d = nc.dram_tensor("me", [C, 512], f32, kind="ExternalInput")
    mo_d = nc.dram_tensor("mo", [C, 512], f32, kind="ExternalInput")
    out_d = nc.dram_tensor("out", [BPC, C], f32, kind="ExternalOutput")

    with tile.TileContext(nc) as tc:
        nc.gpsimd.load_library(library_config.ap_gather)
        with (
            tc.tile_pool(name="weights", bufs=1) as wpool,
            tc.tile_pool(name="compt", bufs=2) as ctpool,
            tc.tile_pool(name="bigS", bufs=2) as spool,
            tc.tile_pool(name="bigV", bufs=1) as vpool,
            tc.tile_pool(name="small", bufs=1) as smpool,
            tc.tile_pool(name="dmain", bufs=4) as scpool,
            tc.tile_pool(name="ps_big", bufs=3, space="PSUM") as ps_big,
            tc.tile_pool(name="ps_a", bufs=1, space="PSUM") as ps_a,
            tc.tile_pool(name="ps_misc", bufs=1, space="PSUM") as ps_m,
        ):
            # ---- weights / constants resident ----
            wq_s = wpool.tile([C, 2048], f32)
            nc.sync.dma_start(wq_s[:], wq_d[:])
            wkT_s = wpool.tile([C, 8 * C], f32)
            nc.sync.dma_start(wkT_s[:], wkT_d[:])
            wv_s = wpool.tile([C, 8 * C], f16)
            nc.sync.dma_start(wv_s[:], wv_d[:])
            wjwp_s = wpool.tile([C, NQ * C], f32)
            nc.sync.dma_start(wjwp_s[:], wjwp_d[:])
            wp_s = wpool.tile([C, C], f32)
            nc.sync.dma_start(wp_s[:], wp_d[:])
            bp4_s = wpool.tile([BPC, C], f32)
            nc.sync.dma_start(bp4_s[:], bp4_d[:])
            hrep_s = wpool.tile([C, C], f32)
            nc.sync.dma_start(hrep_s[:], hrep_d[:])
            choff_s = wpool.tile([C, NCAND], f32)
            nc.sync.dma_start(choff_s[:], choff_d[:])
            me_s = wpool.tile([C, 512], f32)
            nc.sync.dma_start(me_s[:], me_d[:])
            mo_s = wpool.tile([C, 512], f32)
            nc.sync.dma_start(mo_s[:], mo_d[:])
            xT_s = wpool.tile([C, BPC], f32)
            nc.sync.dma_start(xT_s[:], xT_d[:])

            # ---- Q projection, all 4 batches: QT [(h,d), (q,b)] ----
            qt_ps = ps_m.tile([C, 512], f32, tag="misc")
            for q in range(NQ):
                nc.tensor.matmul(
                    qt_ps[:, q * BPC:(q + 1) * BPC],
                    wq_s[:, q * C:(q + 1) * C],
                    xT_s[:],
                )
            qt_s = wpool.tile([C, NQ * BPC], f32)
            nc.scalar.copy(qt_s[:], qt_ps[:, : NQ * BPC])

            pvt4_s = wpool.tile([C, NQ * BPC], f32)   # [(h,d), (q,b)]

            for b in range(BPC):
                # ---- comp_T fp16 hi/lo: host-transposed, DMA straight in ----
                c16h = ctpool.tile([C, NC], f16, tag="c16h")
                nc.sync.dma_start(c16h[:], comphT_d[b])

                # ---- Qbd block-diag with 0.25 scale ----
                qfull_s = smpool.tile([C, C], f32, tag="qfull")
                qsl = (
                    qt_s[:, b::BPC]
                    .rearrange("p (o q) -> p o q", o=1)
                    .to_broadcast([C, H, NQ])
                )
                nc.vector.tensor_scalar(
                    qfull_s[:].rearrange("p (o q) -> p o q", o=H),
                    qsl, 0.25, None, Alu.mult,
                )
                qbd_s = smpool.tile([C, C], f32, tag="qbd")
                nc.vector.tensor_mul(qbd_s[:], qfull_s[:], hrep_s[:])

                # ---- A_e [c,row] fp32 -> fp16 hi/lo ----
                a16h = smpool.tile([C, 8 * C], f16, tag="a16h")
                for half in range(2):
                    a_ps = ps_a.tile([C, 512], f32, tag="a")
                    for i in range(4):
                        e = half * 4 + i
                        nc.tensor.matmul(
                            a_ps[:, i * C:(i + 1) * C],
                            wkT_s[:, e * C:(e + 1) * C],
                            qbd_s[:],
                        )
                    dh = a16h[:, half * 512:(half + 1) * 512]
                    nc.scalar.copy(dh, a_ps[:])

                # ---- S = Ah.Ch + Ah.Cl + Al.Ch (fp16 x3) ----
                sh0 = spool.tile([C, NJ // 2], f32, tag="sh")
                sh1 = spool.tile([C, NJ // 2], f32, tag="sh")
                s_half = [sh0, sh1]
                for e in range(8):
                    ah = a16h[:, e * C:(e + 1) * C]
                    for half in range(2):
                        s_ps = ps_big.tile([C, 1024], f32, tag="big")
                        # single fp16 term: quantization budget covers it
                        for lhs, cc, st, sp in (
                            (ah, c16h, True, True),
                        ):
                            for n in range(2):
                                col = half * 1024 + n * 512
                                nc.tensor.matmul(
                                    s_ps[:, n * 512:(n + 1) * 512],
                                    lhs, cc[:, col:col + 512],
                                    start=st, stop=sp,
                                )
                        sh = s_half[(e * 2 + half) // 8]
                        off = ((e * 2 + half) % 8) * 1024
                        nc.scalar.copy(sh[:, off:off + 1024], s_ps[:])

                # ---- V^T (fp16 inputs, bf16 out) ----
                vt_s = vpool.tile([C, NJ], bf16, tag="VT")
                for e in range(8):
                    for half in range(2):
                        v_ps = ps_big.tile([C, 1024], f32, tag="big")
                        for n in range(2):
                            col = half * 1024 + n * 512
                            nc.tensor.matmul(
                                v_ps[:, n * 512:(n + 1) * 512],
                                wv_s[:, e * C:(e + 1) * C],
                                c16h[:, col:col + 512],
                            )
                        nc.scalar.copy(
                            vt_s[:, e * NC + half * 1024: e * NC + (half + 1) * 1024],
                            v_ps[:],
                        )

                # ---- per-chunk top8 + local indices (chunk=1024) ----
                cand_s = smpool.tile([C, NCAND], f32, tag="cand")
                li_s = smpool.tile([C, NCAND], dt.uint16, tag="li")
                for ch in range(NCH):
                    sh = s_half[ch // 4]
                    sl = sh[:, (ch % 4) * CHUNK:(ch % 4 + 1) * CHUNK]
                    nc.vector.max(cand_s[:, ch * 8:(ch + 1) * 8], sl)
                for ch in range(NCH):
                    sh = s_half[ch // 4]
                    sl = sh[:, (ch % 4) * CHUNK:(ch % 4 + 1) * CHUNK]
                    nc.vector.max_index(
                        li_s[:, ch * 8:(ch + 1) * 8],
                        cand_s[:, ch * 8:(ch + 1) * 8],
                        sl,
                    )

                # ---- exact top-12 marking on cand ----
                t8a = smpool.tile([C, 8], f32, tag="t8a")
                nc.vector.max(t8a[:], cand_s[:])
                c2 = smpool.tile([C, NCAND], f32, tag="c2")
                nc.vector.match_replace(c2[:], t8a[:], cand_s[:], NEG)
                t8b = smpool.tile([C, 8], f32, tag="t8b")
                nc.vector.max(t8b[:], c2[:])
                nx4 = smpool.tile([C, 8], f32, tag="nx4")
                nc.vector.memset(nx4[:], 1e30)
                nc.vector.tensor_copy(nx4[:, 0:4], t8b[:, 0:4])
                rr = smpool.tile([C, NCAND], f32, tag="rr")
                nc.vector.match_replace(rr[:], nx4[:], c2[:], NEG)
                mask12 = smpool.tile([C, NCAND], f32, tag="mask12")
                nc.vector.tensor_scalar(mask12[:], rr[:], -1e29, None, Alu.is_le)

                # ---- pack global_idx*1024 + q10(value); mask; extract ----
                lif = smpool.tile([C, NCAND], f32, tag="lif")
                nc.vector.tensor_copy(lif[:], li_s[:])
                gfl = smpool.tile([C, NCAND], f32, tag="gfl")
                nc.vector.tensor_scalar(gfl[:], lif[:], 1024.0, None, Alu.mult)
                nc.vector.tensor_add(gfl[:], gfl[:], choff_s[:])
                q10 = smpool.tile([C, NCAND], f32, tag="q10")
                nc.vector.tensor_scalar(
                    q10[:], cand_s[:], 4.0, 128.0, Alu.add, Alu.mult
                )
                nc.vector.tensor_scalar(
                    q10[:], q10[:], 1023.0, 1.0, Alu.min, Alu.max
                )
                pm = smpool.tile([C, NCAND], f32, tag="pm")
                nc.vector.tensor_add(pm[:], gfl[:], q10[:])
                nc.vector.tensor_mul(pm[:], pm[:], mask12[:])

                pw = smpool.tile([C, 16], f32, tag="pw")
                nc.vector.max(pw[:, 0:8], pm[:])
                pm2 = smpool.tile([C, NCAND], f32, tag="pm2")
                nc.vector.match_replace(pm2[:], pw[:, 0:8], pm[:], 0.0)
                nc.vector.max(pw[:, 8:16], pm2[:])

                # ---- decode winners: gidx + value -> weights ----
                gidxf = smpool.tile([C, 16], f32, tag="gidxf")
                nc.vector.tensor_scalar(
                    gidxf[:], pw[:], 1.0 / 1024.0, -0.5, Alu.mult, Alu.add
                )
                nc.vector.tensor_scalar(
                    gidxf[:], gidxf[:], MAGIC, MAGIC, Alu.add, Alu.subtract
                )
                vv = smpool.tile([C, 16], f32, tag="vv")
                nc.vector.tensor_scalar(vv[:], gidxf[:], -1024.0, None, Alu.mult)
                nc.vector.tensor_add(vv[:], vv[:], pw[:])
                nc.vector.tensor_scalar(
                    vv[:], vv[:], 1.0 / 128.0, -4.0, Alu.mult, Alu.add
                )
                expv = smpool.tile([C, 16], f32, tag="expv")
                nc.scalar.activation(
                    expv[:], vv[:], mybir.ActivationFunctionType.Exp
                )
                wmask = smpool.tile([C, 16], f32, tag="wmask")
                nc.vector.tensor_scalar(wmask[:], pw[:], 0.5, None, Alu.is_ge)
                wgt = smpool.tile([C, 16], f32, tag="wgt")
                nc.vector.tensor_mul(wgt[:], expv[:], wmask[:])
                den = smpool.tile([C, 1], f32, tag="den")
                nc.vector.tensor_reduce(
                    den[:], wgt[:], mybir.AxisListType.X, Alu.add
                )
                rden = smpool.tile([C, 1], f32, tag="rden")
                nc.vector.reciprocal(rden[:], den[:])
                wn = smpool.tile([C, 16], f32, tag="wn")
                nc.vector.tensor_scalar(wn[:], wgt[:], rden[:], None, Alu.mult)

                # ---- pair index (bf16 gather needs 4B granules: d=2) ----
                gp = smpool.tile([C, 16], f32, tag="gp")
                nc.vector.tensor_scalar(
                    gp[:], gidxf[:], 0.5, -0.25, Alu.mult, Alu.add
                )
                nc.vector.tensor_scalar(
                    gp[:], gp[:], MAGIC, MAGIC, Alu.add, Alu.subtract
                )
                gp_i = smpool.tile([C, 16], dt.int16, tag="gpi")
                nc.vector.tensor_copy(gp_i[:], gp[:])
                par = smpool.tile([C, 16], f32, tag="par")
                nc.vector.tensor_scalar(par[:], gp[:], -2.0, None, Alu.mult)
                nc.vector.tensor_add(par[:], par[:], gidxf[:])
                parc = smpool.tile([C, 16], f32, tag="parc")
                nc.vector.tensor_scalar(
                    parc[:], par[:], -1.0, 1.0, Alu.mult, Alu.add
                )
                wnE = smpool.tile([C, 16], f32, tag="wnE")
                nc.vector.tensor_mul(wnE[:], wn[:], parc[:])
                wnO = smpool.tile([C, 16], f32, tag="wnO")
                nc.vector.tensor_mul(wnO[:], wn[:], par[:])

                # ---- gather V pairs (per-head core lists) ----
                g_s = smpool.tile([C, 512], bf16, tag="G")
                nc.gpsimd.ap_gather(
                    g_s[:], vt_s[:], gp_i[:],
                    channels=C, num_elems=NJ // 2, d=2, num_idxs=256,
                )

                # ---- weights -> [(h,d), (i,q,parity)] via headrep matmul ----
                wEb = (
                    wnE[:].rearrange("p (i o) -> p i o", o=1)
                    .to_broadcast([C, NQ, 32])
                )
                wOb = (
                    wnO[:].rearrange("p (i o) -> p i o", o=1)
                    .to_broadcast([C, NQ, 32])
                )
                tmpE = smpool.tile([C, 512], f32, tag="tmpE")
                nc.vector.tensor_mul(
                    tmpE[:].rearrange("p (i s) -> p i s", s=32),
                    wEb,
                    me_s[:].rearrange("p (i s) -> p i s", s=32),
                )
                wsc = smpool.tile([C, 512], f32, tag="wsc")
                nc.vector.tensor_mul(
                    wsc[:].rearrange("p (i s) -> p i s", s=32),
                    wOb,
                    mo_s[:].rearrange("p (i s) -> p i s", s=32),
                )
                nc.vector.tensor_add(wsc[:], wsc[:], tmpE[:])
                wb_ps = ps_m.tile([C, 512], f32, tag="misc")
                nc.tensor.matmul(wb_ps[:], hrep_s[:], wsc[:])
                wb_s = smpool.tile([C, 512], bf16, tag="wb")
                nc.scalar.copy(wb_s[:], wb_ps[:])

                gw = smpool.tile([C, 512], f32, tag="gw")
                nc.vector.tensor_mul(gw[:], g_s[:], wb_s[:])
                # reduce over (i, parity), keep q: write PV^T into (q,b) cols
                nc.vector.tensor_reduce(
                    pvt4_s[:, b::BPC],
                    gw[:].rearrange("p (i q r) -> p q i r", q=NQ, r=2),
                    mybir.AxisListType.XY,
                    Alu.add,
                )

            # ---- final projections for all 4 batches ----
            o1_ps = ps_m.tile([C, 512], f32, tag="misc")
            for q in range(NQ):
                nc.tensor.matmul(
                    o1_ps[:, 0:BPC],
                    wjwp_s[:, q * C:(q + 1) * C],
                    pvt4_s[:, q * BPC:(q + 1) * BPC],
                    start=(q == 0),
                    stop=(q == NQ - 1),
                )
            o2_s = smpool.tile([C, BPC], f32, tag="o2")
            nc.vector.tensor_add(o2_s[:], o1_ps[:, 0:BPC], xT_s[:])
            o3_ps = ps_m.tile([C, 512], f32, tag="misc")
            nc.tensor.matmul(o3_ps[0:BPC, 0:C], o2_s[:], wp_s[:])
            o4_s = smpool.tile([BPC, C], f32, tag="o4")
            nc.vector.tensor_add(o4_s[:], o3_ps[0:BPC, 0:C], bp4_s[:])
            nc.sync.dma_start(out_d[:], o4_s[:])

    nc.compile()
    return nc


def _host_prep(inputs):
    x = np.asarray(inputs["x"], dtype=np.float32)              # [32, 1, 128]
    complement = np.asarray(inputs["complement"], np.float32)  # [32, 2047, 128]
    Wq = np.asarray(inputs["Wq"], np.float32)
    Wkv = np.asarray(inputs["Wkv"], np.float32)
    Wjw = np.asarray(inputs["Wjw"], np.float32)
    Wp = np.asarray(inputs["Wp"], np.float32)
    bp = np.asarray(inputs["bp"], np.float32)

    wkT = np.empty((C, 8 * C), np.float32)
    wv = np.empty((C, 8 * C), np.float32)
    for e in range(8):
        wkT[:, e * C:(e + 1) * C] = Wkv[:, e * 256: e * 256 + 128].T
        wv[:, e * C:(e + 1) * C] = Wkv[:, e * 256 + 128: e * 256 + 256]
    wv = wv.astype(np.float16)
    # Wjw rows are (h,q,d); per-q slice with rows (h,d)
    wjwp = (
        Wjw.reshape(H, NQ, HD, C).transpose(1, 0, 2, 3).reshape(NQ, C, C)
        .transpose(1, 0, 2).reshape(C, NQ * C)
    )
    bp4 = np.tile(bp.reshape(1, C), (BPC, 1)).astype(np.float32)
    hrep = np.kron(np.eye(H, dtype=np.float32), np.ones((HD, HD), np.float32))
    choffrow = ((np.arange(NCAND) // 8) * (CHUNK * 1024)).astype(np.float32)
    choff = np.tile(choffrow.reshape(1, NCAND), (C, 1))
    s_idx = np.tile(np.arange(32).reshape(1, 1, 32), (C, NQ, 1))
    p_idx = (np.arange(C) % NQ).reshape(C, 1, 1)
    me = (s_idx == 2 * p_idx).astype(np.float32).reshape(C, 512)
    mo = (s_idx == 2 * p_idx + 1).astype(np.float32).reshape(C, 512)

    shared = dict(
        wq=np.ascontiguousarray(Wq),
        wkT=np.ascontiguousarray(wkT),
        wv=np.ascontiguousarray(wv),
        wjwp=np.ascontiguousarray(wjwp),
        wp=np.ascontiguousarray(Wp),
        bp4=bp4,
        hrep=np.ascontiguousarray(hrep),
        choff=np.ascontiguousarray(choff),
        me=np.ascontiguousarray(me),
        mo=np.ascontiguousarray(mo),
    )

    in_maps = []
    for core in range(CORES):
        bs = range(core * BPC, (core + 1) * BPC)
        comp = np.stack(
            [
                np.concatenate([x[b].reshape(1, C), complement[b]], axis=0)
                for b in bs
            ]
        ).astype(np.float32)
        compT = comp.transpose(0, 2, 1)          # [BPC, C, NC]
        comphT = compT.astype(np.float16)
        complT = (compT - comphT.astype(np.float32)).astype(np.float16)
        xT = np.ascontiguousarray(x[list(bs)].reshape(BPC, C).T)
        m = dict(shared)
        m["comphT"] = np.ascontiguousarray(comphT)
        m["complT"] = np.ascontiguousarray(complT)
        m["xT"] = xT
        in_maps.append(m)
    return in_maps


def kernel(**inputs):
    from concourse.bass_utils import run_bass_kernel_spmd

    if "prog" not in _prog_cache:
        _prog_cache["prog"] = _build_program()
    nc = _prog_cache["prog"]

    in_maps = _host_prep(inputs)
    res = run_bass_kernel_spmd(nc, in_maps, core_ids=list(range(CORES)))
    out = np.empty((B, 1, C), np.float32)
    for core in range(CORES):
        o = res.results[core]["out"]
        for i in range(BPC):
            out[core * BPC + i, 0, :] = o[i]
    return out


if __name__ == "__main__":
    d = np.load("/root/problem/inputs_cache.npz")
    inputs = {k: d[k] for k in d.files}
    got = kernel(**inputs)
    print("kernel output:", got.shape, got.dtype, np.abs(got).max())



# revision 19
# speedup vs baseline: 1.0841x; 1.0841x over previous
"""Trainium2 Bass kernel for nn_MultiHeadCrossAttention (B=32, Nc=2048, H=8, topk=12).

kernel(**inputs) takes FULL inputs, returns FULL output [32, 1, 128].
Batch is sharded 4-per-core across 8 NeuronCores (data parallel, no collectives).

Per-batch device algorithm (rows=(h,q) 128 wide, j = e*2048+nc in [0,16384)):
  comp_T fp16 (single term: logit error ~1e-4 << q10 weight quantum 1/128)
  A_e[c,row]  = WkT_e.T @ Qbd -> fp16
  S_e[row,nc] = Ah.T@Ch   (single fp16 matmul per chunk)
  VT_e[hd,nc] = Wv_e.T @ Ch -> VT [128,16384] bf16
  per-chunk(1024) top8 (DVE max8) -> cand [128,128]
  per-chunk max_index -> local indices
  exact global top-12 marking via max8/match_replace rounds on cand
  pack (global_idx*1024 + quantized_value), extract winners via max8
  weights = exp(value)/sum   (selection exact; weight quantization ~0.4%)
  G = ap_gather(VT pairs, winner idx/2; 16-partition cores align with heads)
  PV^T[(h,d),q] = sum w*G  (headrep matmul broadcasts weights, parity split)
  out = (PV flat @ WjwP) + x;  out = out @ Wp + bp
"""

import sys
import numpy as np

for p in ("/opt/trn_rl_repo",):
    if p not in sys.path:
        sys.path.insert(0, p)

import ml_dtypes

B, CORES, BPC = 32, 8, 4
H, HD, NQ, TK, C, NC = 8, 16, 16, 12, 128, 2048
NJ = 8 * NC            # 16384
CHUNK = 1024
NCH = NJ // CHUNK      # 16
NCAND = NCH * 8        # 128
NEG = -1e30
MAGIC = 12582912.0     # 2**23 + 2**22: add/sub rounds fp32 to nearest int

_prog_cache = {}


def _build_program():
    import concourse.bass as bass
    import concourse.mybir as mybir
    import concourse.tile as tile
    from concourse import bacc
    from concourse import library_config

    dt = mybir.dt
    Alu = mybir.AluOpType
    f32, f16, bf16 = dt.float32, dt.float16, dt.bfloat16
    nc = bacc.Bacc("TRN2", target_bir_lowering=False)

    comphT_d = nc.dram_tensor("comphT", [BPC, C, NC], f16, kind="ExternalInput")
    complT_d = nc.dram_tensor("complT", [BPC, C, NC], f16, kind="ExternalInput")
    xT_d = nc.dram_tensor("xT", [C, BPC], f32, kind="ExternalInput")
    wq_d = nc.dram_tensor("wq", [C, 2048], f32, kind="ExternalInput")
    wkT_d = nc.dram_tensor("wkT", [C, 8 * C], f32, kind="ExternalInput")
    wv_d = nc.dram_tensor("wv", [C, 8 * C], f16, kind="ExternalInput")
    wjwp_d = nc.dram_tensor("wjwp", [C, NQ * C], f32, kind="ExternalInput")
    wp_d = nc.dram_tensor("wp", [C, C], f32, kind="ExternalInput")
    bp4_d = nc.dram_tensor("bp4", [BPC, C], f32, kind="ExternalInput")
    hrep_d = nc.dram_tensor("hrep", [C, C], f32, kind="ExternalInput")
    choff_d = nc.dram_tensor("choff", [C, NCAND], f32, kind="ExternalInput")
    me_d = nc.dram_tensor("me", [C, 512], f32, kind="ExternalInput")
    mo_d = nc.dram_tensor("mo", [C, 512], f32, kind="ExternalInput")
    out_d = nc.dram_tensor("out", [BPC, C], f32, kind="ExternalOutput")

    with tile.TileContext(nc) as tc:
        nc.gpsimd.load_library(library_config.ap_gather)
        with (
            tc.tile_pool(name="weights", bufs=1) as wpool,
            tc.tile_pool(name="compt", bufs=2) as ctpool,
            tc.tile_pool(name="bigS", bufs=2) as spool,
            tc.tile_pool(name="bigV", bufs=1) as vpool,
            tc.tile_pool(name="small", bufs=1) as smpool,
            tc.tile_pool(name="dmain", bufs=4) as scpool,
            tc.tile_pool(name="ps_big", bufs=3, space="PSUM") as ps_big,
            tc.tile_pool(name="ps_a", bufs=1, space="PSUM") as ps_a,
            tc.tile_pool(name="ps_misc", bufs=1, space="PSUM") as ps_m,
        ):
            # ---- weights / constants resident ----
            wq_s = wpool.tile([C, 2048], f32)
            nc.sync.dma_start(wq_s[:], wq_d[:])
            wkT_s = wpool.tile([C, 8 * C], f32)
            nc.sync.dma_start(wkT_s[:], wkT_d[:])
            wv_s = wpool.tile([C, 8 * C], f16)
            nc.sync.dma_start(wv_s[:], wv_d[:])
            wjwp_s = wpool.tile([C, NQ * C], f32)
            wp_s = wpool.tile([C, C], f32)
            bp4_s = wpool.tile([BPC, C], f32)
            hrep_s = wpool.tile([C, C], f32)
            nc.sync.dma_start(hrep_s[:], hrep_d[:])
            choff_s = wpool.tile([C, NCAND], f32)
            nc.sync.dma_start(choff_s[:], choff_d[:])
            me_s = wpool.tile([C, 512], f32)
            nc.sync.dma_start(me_s[:], me_d[:])
            mo_s = wpool.tile([C, 512], f32)
            nc.sync.dma_start(mo_s[:], mo_d[:])
            xT_s = wpool.tile([C, BPC], f32)
            nc.sync.dma_start(xT_s[:], xT_d[:])

            # ---- Q projection, all 4 batches: QT [(h,d), (q,b)] ----
            qt_ps = ps_m.tile([C, 512], f32, tag="misc")
            for q in range(NQ):
                nc.tensor.matmul(
                    qt_ps[:, q * BPC:(q + 1) * BPC],
                    wq_s[:, q * C:(q + 1) * C],
                    xT_s[:],
                )
            qt_s = wpool.tile([C, NQ * BPC], f32)
            nc.scalar.copy(qt_s[:], qt_ps[:, : NQ * BPC])

            pvt4_s = wpool.tile([C, NQ * BPC], f32)   # [(h,d), (q,b)]

            # late-use weights: issued after early DMAs so batch-0 comp
            # transfer is not queued behind them
            nc.sync.dma_start(wjwp_s[:], wjwp_d[:])
            nc.sync.dma_start(wp_s[:], wp_d[:])
            nc.sync.dma_start(bp4_s[:], bp4_d[:])

            for b in range(BPC):
                # ---- comp_T fp16 hi/lo: host-transposed, DMA straight in ----
                c16h = ctpool.tile([C, NC], f16, tag="c16h")
                nc.sync.dma_start(c16h[:], comphT_d[b])

                # ---- Qbd block-diag with 0.25 scale ----
                qfull_s = smpool.tile([C, C], f32, tag="qfull")
                qsl = (
                    qt_s[:, b::BPC]
                    .rearrange("p (o q) -> p o q", o=1)
                    .to_broadcast([C, H, NQ])
                )
                nc.vector.tensor_scalar(
                    qfull_s[:].rearrange("p (o q) -> p o q", o=H),
                    qsl, 0.25, None, Alu.mult,
                )
                qbd_s = smpool.tile([C, C], f32, tag="qbd")
                nc.vector.tensor_mul(qbd_s[:], qfull_s[:], hrep_s[:])

                # ---- A_e [c,row] fp32 -> fp16 hi/lo ----
                a16h = smpool.tile([C, 8 * C], f16, tag="a16h")
                for half in range(2):
                    a_ps = ps_a.tile([C, 512], f32, tag="a")
                    for i in range(4):
                        e = half * 4 + i
                        nc.tensor.matmul(
                            a_ps[:, i * C:(i + 1) * C],
                            wkT_s[:, e * C:(e + 1) * C],
                            qbd_s[:],
                        )
                    dh = a16h[:, half * 512:(half + 1) * 512]
                    nc.scalar.copy(dh, a_ps[:])

                # ---- S = Ah.Ch + Ah.Cl + Al.Ch (fp16 x3) ----
                sh0 = spool.tile([C, NJ // 2], f32, tag="sh")
                sh1 = spool.tile([C, NJ // 2], f32, tag="sh")
                s_half = [sh0, sh1]
                for e in range(8):
                    ah = a16h[:, e * C:(e + 1) * C]
                    for half in range(2):
                        s_ps = ps_big.tile([C, 1024], f32, tag="big")
                        # single fp16 term: quantization budget covers it
                        for lhs, cc, st, sp in (
                            (ah, c16h, True, True),
                        ):
                            for n in range(2):
                                col = half * 1024 + n * 512
                                nc.tensor.matmul(
                                    s_ps[:, n * 512:(n + 1) * 512],
                                    lhs, cc[:, col:col + 512],
                                    start=st, stop=sp,
                                )
                        sh = s_half[(e * 2 + half) // 8]
                        off = ((e * 2 + half) % 8) * 1024
                        nc.scalar.copy(sh[:, off:off + 1024], s_ps[:])

                # ---- V^T (fp16 inputs, bf16 out) ----
                vt_s = vpool.tile([C, NJ], bf16, tag="VT")
                for e in range(8):
                    for half in range(2):
                        v_ps = ps_big.tile([C, 1024], f32, tag="big")
                        for n in range(2):
                            col = half * 1024 + n * 512
                            nc.tensor.matmul(
                                v_ps[:, n * 512:(n + 1) * 512],
                                wv_s[:, e * C:(e + 1) * C],
                                c16h[:, col:col + 512],
                            )
                        nc.scalar.copy(
                            vt_s[:, e * NC + half * 1024: e * NC + (half + 1) * 1024],
                            v_ps[:],
                        )

                # ---- per-chunk top8 + local indices (chunk=1024) ----
                cand_s = smpool.tile([C, NCAND], f32, tag="cand")
                li_s = smpool.tile([C, NCAND], dt.uint16, tag="li")
                for ch in range(NCH):
                    sh = s_half[ch // 8]
                    sl = sh[:, (ch % 8) * CHUNK:(ch % 8 + 1) * CHUNK]
                    nc.vector.max(cand_s[:, ch * 8:(ch + 1) * 8], sl)
                for ch in range(NCH):
                    sh = s_half[ch // 8]
                    sl = sh[:, (ch % 8) * CHUNK:(ch % 8 + 1) * CHUNK]
                    nc.vector.max_index(
                        li_s[:, ch * 8:(ch + 1) * 8],
                        cand_s[:, ch * 8:(ch + 1) * 8],
                        sl,
                    )

                # ---- exact top-12 marking on cand ----
                t8a = smpool.tile([C, 8], f32, tag="t8a")
                nc.vector.max(t8a[:], cand_s[:])
                c2 = smpool.tile([C, NCAND], f32, tag="c2")
                nc.vector.match_replace(c2[:], t8a[:], cand_s[:], NEG)
                t8b = smpool.tile([C, 8], f32, tag="t8b")
                nc.vector.max(t8b[:], c2[:])
                nx4 = smpool.tile([C, 8], f32, tag="nx4")
                nc.vector.memset(nx4[:], 1e30)
                nc.vector.tensor_copy(nx4[:, 0:4], t8b[:, 0:4])
                rr = smpool.tile([C, NCAND], f32, tag="rr")
                nc.vector.match_replace(rr[:], nx4[:], c2[:], NEG)
                mask12 = smpool.tile([C, NCAND], f32, tag="mask12")
                nc.vector.tensor_scalar(mask12[:], rr[:], -1e29, None, Alu.is_le)

                # ---- pack global_idx*1024 + q10(value); mask; extract ----
                lif = smpool.tile([C, NCAND], f32, tag="lif")
                nc.vector.tensor_copy(lif[:], li_s[:])
                gfl = smpool.tile([C, NCAND], f32, tag="gfl")
                nc.vector.tensor_scalar(gfl[:], lif[:], 1024.0, None, Alu.mult)
                nc.vector.tensor_add(gfl[:], gfl[:], choff_s[:])
                q10 = smpool.tile([C, NCAND], f32, tag="q10")
                nc.vector.tensor_scalar(
                    q10[:], cand_s[:], 4.0, 128.0, Alu.add, Alu.mult
                )
                nc.vector.tensor_scalar(
                    q10[:], q10[:], 1023.0, 1.0, Alu.min, Alu.max
                )
                pm = smpool.tile([C, NCAND], f32, tag="pm")
                nc.vector.tensor_add(pm[:], gfl[:], q10[:])
                nc.vector.tensor_mul(pm[:], pm[:], mask12[:])

                pw = smpool.tile([C, 16], f32, tag="pw")
                nc.vector.max(pw[:, 0:8], pm[:])
                pm2 = smpool.tile([C, NCAND], f32, tag="pm2")
                nc.vector.match_replace(pm2[:], pw[:, 0:8], pm[:], 0.0)
                nc.vector.max(pw[:, 8:16], pm2[:])

                # ---- decode winners: gidx + value -> weights ----
                gidxf = smpool.tile([C, 16], f32, tag="gidxf")
                nc.vector.tensor_scalar(
                    gidxf[:], pw[:], 1.0 / 1024.0, -0.5, Alu.mult, Alu.add
                )
                nc.vector.tensor_scalar(
                    gidxf[:], gidxf[:], MAGIC, MAGIC, Alu.add, Alu.subtract
                )
                vv = smpool.tile([C, 16], f32, tag="vv")
                nc.vector.tensor_scalar(vv[:], gidxf[:], -1024.0, None, Alu.mult)
                nc.vector.tensor_add(vv[:], vv[:], pw[:])
                nc.vector.tensor_scalar(
                    vv[:], vv[:], 1.0 / 128.0, -4.0, Alu.mult, Alu.add
                )
                expv = smpool.tile([C, 16], f32, tag="expv")
                nc.scalar.activation(
                    expv[:], vv[:], mybir.ActivationFunctionType.Exp
                )
                wmask = smpool.tile([C, 16], f32, tag="wmask")
                nc.vector.tensor_scalar(wmask[:], pw[:], 0.5, None, Alu.is_ge)
                wgt = smpool.tile([C, 16], f32, tag="wgt")
                nc.vector.tensor_mul(wgt[:], expv[:], wmask[:])
                den = smpool.tile([C, 1], f32, tag="den")
                nc.vector.tensor_reduce(
                    den[:], wgt[:], mybir.AxisListType.X, Alu.add
                )
                rden = smpool.tile([C, 1], f32, tag="rden")
                nc.vector.reciprocal(rden[:], den[:])
                wn = smpool.tile([C, 16], f32, tag="wn")
                nc.vector.tensor_scalar(wn[:], wgt[:], rden[:], None, Alu.mult)

                # ---- pair index (bf16 gather needs 4B granules: d=2) ----
                gp = smpool.tile([C, 16], f32, tag="gp")
                nc.vector.tensor_scalar(
                    gp[:], gidxf[:], 0.5, -0.25, Alu.mult, Alu.add
                )
                nc.vector.tensor_scalar(
                    gp[:], gp[:], MAGIC, MAGIC, Alu.add, Alu.subtract
                )
                gp_i = smpool.tile([C, 16], dt.int16, tag="gpi")
                nc.vector.tensor_copy(gp_i[:], gp[:])
                par = smpool.tile([C, 16], f32, tag="par")
                nc.vector.tensor_scalar(par[:], gp[:], -2.0, None, Alu.mult)
                nc.vector.tensor_add(par[:], par[:], gidxf[:])
                parc = smpool.tile([C, 16], f32, tag="parc")
                nc.vector.tensor_scalar(
                    parc[:], par[:], -1.0, 1.0, Alu.mult, Alu.add
                )
                wnE = smpool.tile([C, 16], f32, tag="wnE")
                nc.vector.tensor_mul(wnE[:], wn[:], parc[:])
                wnO = smpool.tile([C, 16], f32, tag="wnO")
                nc.vector.tensor_mul(wnO[:], wn[:], par[:])

                # ---- gather V pairs (per-head core lists) ----
                g_s = smpool.tile([C, 512], bf16, tag="G")
                nc.gpsimd.ap_gather(
                    g_s[:], vt_s[:], gp_i[:],
                    channels=C, num_elems=NJ // 2, d=2, num_idxs=256,
                )

                # ---- weights -> [(h,d), (i,q,parity)] via headrep matmul ----
                wEb = (
                    wnE[:].rearrange("p (i o) -> p i o", o=1)
                    .to_broadcast([C, NQ, 32])
                )
                wOb = (
                    wnO[:].rearrange("p (i o) -> p i o", o=1)
                    .to_broadcast([C, NQ, 32])
                )
                tmpE = smpool.tile([C, 512], f32, tag="tmpE")
                nc.vector.tensor_mul(
                    tmpE[:].rearrange("p (i s) -> p i s", s=32),
                    wEb,
                    me_s[:].rearrange("p (i s) -> p i s", s=32),
                )
                wsc = smpool.tile([C, 512], f32, tag="wsc")
                nc.vector.tensor_mul(
                    wsc[:].rearrange("p (i s) -> p i s", s=32),
                    wOb,
                    mo_s[:].rearrange("p (i s) -> p i s", s=32),
                )
                nc.vector.tensor_add(wsc[:], wsc[:], tmpE[:])
                wb_ps = ps_m.tile([C, 512], f32, tag="misc")
                nc.tensor.matmul(wb_ps[:], hrep_s[:], wsc[:])
                wb_s = smpool.tile([C, 512], bf16, tag="wb")
                nc.scalar.copy(wb_s[:], wb_ps[:])

                gw = smpool.tile([C, 512], f32, tag="gw")
                nc.vector.tensor_mul(gw[:], g_s[:], wb_s[:])
                # reduce over (i, parity), keep q: write PV^T into (q,b) cols
                nc.vector.tensor_reduce(
                    pvt4_s[:, b::BPC],
                    gw[:].rearrange("p (i q r) -> p q i r", q=NQ, r=2),
                    mybir.AxisListType.XY,
                    Alu.add,
                )

            # ---- final projections for all 4 batches ----
            o1_ps = ps_m.tile([C, 512], f32, tag="misc")
            for q in range(NQ):
                nc.tensor.matmul(
                    o1_ps[:, 0:BPC],
                    wjwp_s[:, q * C:(q + 1) * C],
                    pvt4_s[:, q * BPC:(q + 1) * BPC],
                    start=(q == 0),
                    stop=(q == NQ - 1),
                )
            o2_s = smpool.tile([C, BPC], f32, tag="o2")
            nc.vector.tensor_add(o2_s[:], o1_ps[:, 0:BPC], xT_s[:])
            o3_ps = ps_m.tile([C, 512], f32, tag="misc")
            nc.tensor.matmul(o3_ps[0:BPC, 0:C], o2_s[:], wp_s[:])
            o4_s = smpool.tile([BPC, C], f32, tag="o4")
            nc.vector.tensor_add(o4_s[:], o3_ps[0:BPC, 0:C], bp4_s[:])
            nc.sync.dma_start(out_d[:], o4_s[:])

    nc.compile()
    return nc


def _host_prep(inputs):
    x = np.asarray(inputs["x"], dtype=np.float32)              # [32, 1, 128]
    complement = np.asarray(inputs["complement"], np.float32)  # [32, 2047, 128]
    Wq = np.asarray(inputs["Wq"], np.float32)
    Wkv = np.asarray(inputs["Wkv"], np.float32)
    Wjw = np.asarray(inputs["Wjw"], np.float32)
    Wp = np.asarray(inputs["Wp"], np.float32)
    bp = np.asarray(inputs["bp"], np.float32)

    wkT = np.empty((C, 8 * C), np.float32)
    wv = np.empty((C, 8 * C), np.float32)
    for e in range(8):
        wkT[:, e * C:(e + 1) * C] = Wkv[:, e * 256: e * 256 + 128].T
        wv[:, e * C:(e + 1) * C] = Wkv[:, e * 256 + 128: e * 256 + 256]
    wv = wv.astype(np.float16)
    # Wjw rows are (h,q,d); per-q slice with rows (h,d)
    wjwp = (
        Wjw.reshape(H, NQ, HD, C).transpose(1, 0, 2, 3).reshape(NQ, C, C)
        .transpose(1, 0, 2).reshape(C, NQ * C)
    )
    bp4 = np.tile(bp.reshape(1, C), (BPC, 1)).astype(np.float32)
    hrep = np.kron(np.eye(H, dtype=np.float32), np.ones((HD, HD), np.float32))
    choffrow = ((np.arange(NCAND) // 8) * (CHUNK * 1024)).astype(np.float32)
    choff = np.tile(choffrow.reshape(1, NCAND), (C, 1))
    s_idx = np.tile(np.arange(32).reshape(1, 1, 32), (C, NQ, 1))
    p_idx = (np.arange(C) % NQ).reshape(C, 1, 1)
    me = (s_idx == 2 * p_idx).astype(np.float32).reshape(C, 512)
    mo = (s_idx == 2 * p_idx + 1).astype(np.float32).reshape(C, 512)

    shared = dict(
        wq=np.ascontiguousarray(Wq),
        wkT=np.ascontiguousarray(wkT),
        wv=np.ascontiguousarray(wv),
        wjwp=np.ascontiguousarray(wjwp),
        wp=np.ascontiguousarray(Wp),
        bp4=bp4,
        hrep=np.ascontiguousarray(hrep),
        choff=np.ascontiguousarray(choff),
        me=np.ascontiguousarray(me),
        mo=np.ascontiguousarray(mo),
    )

    in_maps = []
    for core in range(CORES):
        bs = range(core * BPC, (core + 1) * BPC)
        comp = np.stack(
            [
                np.concatenate([x[b].reshape(1, C), complement[b]], axis=0)
                for b in bs
            ]
        ).astype(np.float32)
        compT = comp.transpose(0, 2, 1)          # [BPC, C, NC]
        comphT = compT.astype(np.float16)
        complT = (compT - comphT.astype(np.float32)).astype(np.float16)
        xT = np.ascontiguousarray(x[list(bs)].reshape(BPC, C).T)
        m = dict(shared)
        m["comphT"] = np.ascontiguousarray(comphT)
        m["complT"] = np.ascontiguousarray(complT)
        m["xT"] = xT
        in_maps.append(m)
    return in_maps


def kernel(**inputs):
    from concourse.bass_utils import run_bass_kernel_spmd

    if "prog" not in _prog_cache:
        _prog_cache["prog"] = _build_program()
    nc = _prog_cache["prog"]

    in_maps = _host_prep(inputs)
    res = run_bass_kernel_spmd(nc, in_maps, core_ids=list(range(CORES)))
    out = np.empty((B, 1, C), np.float32)
    for core in range(CORES):
        o = res.results[core]["out"]
        for i in range(BPC):
            out[core * BPC + i, 0, :] = o[i]
    return out


if __name__ == "__main__":
    d = np.load("/root/problem/inputs_cache.npz")
    inputs = {k: d[k] for k in d.files}
    got = kernel(**inputs)
    print("kernel output:", got.shape, got.dtype, np.abs(got).max())



# revision 20
# speedup vs baseline: 1.0923x; 1.0076x over previous
"""Trainium2 Bass kernel for nn_MultiHeadCrossAttention (B=32, Nc=2048, H=8, topk=12).

kernel(**inputs) takes FULL inputs, returns FULL output [32, 1, 128].
Batch is sharded 4-per-core across 8 NeuronCores (data parallel, no collectives).

Per-batch device algorithm (rows=(h,q) 128 wide, j = e*2048+nc in [0,16384)):
  comp_T fp16 (single term: logit error ~1e-4 << q10 weight quantum 1/128)
  A_e[c,row]  = WkT_e.T @ Qbd -> fp16
  S_e[row,nc] = Ah.T@Ch   (single fp16 matmul per chunk)
  VT_e[hd,nc] = Wv_e.T @ Ch -> VT [128,16384] bf16
  per-chunk(1024) top8 (DVE max8) -> cand [128,128]
  per-chunk max_index -> local indices
  exact global top-12 marking via max8/match_replace rounds on cand
  pack (global_idx*1024 + quantized_value), extract winners via max8
  weights = exp(value)/sum   (selection exact; weight quantization ~0.4%)
  G = ap_gather(VT pairs, winner idx/2; 16-partition cores align with heads)
  PV^T[(h,d),q] = sum w*G  (headrep matmul broadcasts weights, parity split)
  out = (PV flat @ WjwP) + x;  out = out @ Wp + bp
"""

import sys
import numpy as np

for p in ("/opt/trn_rl_repo",):
    if p not in sys.path:
        sys.path.insert(0, p)

import ml_dtypes

B, CORES, BPC = 32, 8, 4
H, HD, NQ, TK, C, NC = 8, 16, 16, 12, 128, 2048
NJ = 8 * NC            # 16384
CHUNK = 1024
NCH = NJ // CHUNK      # 16
NCAND = NCH * 8        # 128
NEG = -1e30
MAGIC = 12582912.0     # 2**23 + 2**22: add/sub rounds fp32 to nearest int

_prog_cache = {}


def _build_program():
    import concourse.bass as bass
    import concourse.mybir as mybir
    import concourse.tile as tile
    from concourse import bacc
    from concourse import library_config

    dt = mybir.dt
    Alu = mybir.AluOpType
    f32, f16, bf16 = dt.float32, dt.float16, dt.bfloat16
    nc = bacc.Bacc("TRN2", target_bir_lowering=False)

    comphT_d = nc.dram_tensor("comphT", [BPC, C, NC], f16, kind="ExternalInput")
    complT_d = nc.dram_tensor("complT", [BPC, C, NC], f16, kind="ExternalInput")
    xT_d = nc.dram_tensor("xT", [C, BPC], f32, kind="ExternalInput")
    wq_d = nc.dram_tensor("wq", [C, 2048], f32, kind="ExternalInput")
    wkT_d = nc.dram_tensor("wkT", [C, 8 * C], f32, kind="ExternalInput")
    wv_d = nc.dram_tensor("wv", [C, 8 * C], f16, kind="ExternalInput")
    wjwp_d = nc.dram_tensor("wjwp", [C, NQ * C], f32, kind="ExternalInput")
    wp_d = nc.dram_tensor("wp", [C, C], f32, kind="ExternalInput")
    bp4_d = nc.dram_tensor("bp4", [BPC, C], f32, kind="ExternalInput")
    hrep_d = nc.dram_tensor("hrep", [C, C], f32, kind="ExternalInput")
    choff_d = nc.dram_tensor("choff", [C, NCAND], f32, kind="ExternalInput")
    me_d = nc.dram_tensor("me", [C, 512], f32, kind="ExternalInput")
    mo_d = nc.dram_tensor("mo", [C, 512], f32, kind="ExternalInput")
    out_d = nc.dram_tensor("out", [BPC, C], f32, kind="ExternalOutput")

    with tile.TileContext(nc) as tc:
        nc.gpsimd.load_library(library_config.ap_gather)
        with (
            tc.tile_pool(name="weights", bufs=1) as wpool,
            tc.tile_pool(name="compt", bufs=2) as ctpool,
            tc.tile_pool(name="bigS", bufs=2) as spool,
            tc.tile_pool(name="bigV", bufs=1) as vpool,
            tc.tile_pool(name="small", bufs=1) as smpool,
            tc.tile_pool(name="dmain", bufs=4) as scpool,
            tc.tile_pool(name="ps_big", bufs=3, space="PSUM") as ps_big,
            tc.tile_pool(name="ps_a", bufs=1, space="PSUM") as ps_a,
            tc.tile_pool(name="ps_misc", bufs=1, space="PSUM") as ps_m,
        ):
            # ---- weights / constants resident (early-use first) ----
            xT_s = wpool.tile([C, BPC], f32)
            nc.sync.dma_start(xT_s[:], xT_d[:])
            wq_s = wpool.tile([C, 2048], f32)
            nc.sync.dma_start(wq_s[:], wq_d[:])
            hrep_s = wpool.tile([C, C], f32)
            nc.sync.dma_start(hrep_s[:], hrep_d[:])
            wkT_s = wpool.tile([C, 8 * C], f32)
            nc.sync.dma_start(wkT_s[:], wkT_d[:])
            wv_s = wpool.tile([C, 8 * C], f16)
            nc.sync.dma_start(wv_s[:], wv_d[:])
            wjwp_s = wpool.tile([C, NQ * C], f32)
            wp_s = wpool.tile([C, C], f32)
            bp4_s = wpool.tile([BPC, C], f32)
            choff_s = wpool.tile([C, NCAND], f32)
            me_s = wpool.tile([C, 512], f32)
            mo_s = wpool.tile([C, 512], f32)

            # ---- Q projection, all 4 batches: QT [(h,d), (q,b)] ----
            qt_ps = ps_m.tile([C, 512], f32, tag="misc")
            for q in range(NQ):
                nc.tensor.matmul(
                    qt_ps[:, q * BPC:(q + 1) * BPC],
                    wq_s[:, q * C:(q + 1) * C],
                    xT_s[:],
                )
            qt_s = wpool.tile([C, NQ * BPC], f32)
            nc.scalar.copy(qt_s[:], qt_ps[:, : NQ * BPC])

            pvt4_s = wpool.tile([C, NQ * BPC], f32)   # [(h,d), (q,b)]

            # late-use weights: issued after early DMAs so batch-0 comp
            # transfer is not queued behind them
            nc.sync.dma_start(choff_s[:], choff_d[:])
            nc.sync.dma_start(me_s[:], me_d[:])
            nc.sync.dma_start(mo_s[:], mo_d[:])
            nc.sync.dma_start(wjwp_s[:], wjwp_d[:])
            nc.sync.dma_start(wp_s[:], wp_d[:])
            nc.sync.dma_start(bp4_s[:], bp4_d[:])

            for b in range(BPC):
                # ---- comp_T fp16 hi/lo: host-transposed, DMA straight in ----
                c16h = ctpool.tile([C, NC], f16, tag="c16h")
                nc.sync.dma_start(c16h[:], comphT_d[b])

                # ---- Qbd block-diag with 0.25 scale ----
                qfull_s = smpool.tile([C, C], f32, tag="qfull")
                qsl = (
                    qt_s[:, b::BPC]
                    .rearrange("p (o q) -> p o q", o=1)
                    .to_broadcast([C, H, NQ])
                )
                nc.vector.tensor_scalar(
                    qfull_s[:].rearrange("p (o q) -> p o q", o=H),
                    qsl, 0.25, None, Alu.mult,
                )
                qbd_s = smpool.tile([C, C], f32, tag="qbd")
                nc.vector.tensor_mul(qbd_s[:], qfull_s[:], hrep_s[:])

                # ---- A_e [c,row] fp32 -> fp16 hi/lo ----
                a16h = smpool.tile([C, 8 * C], f16, tag="a16h")
                for half in range(2):
                    a_ps = ps_a.tile([C, 512], f32, tag="a")
                    for i in range(4):
                        e = half * 4 + i
                        nc.tensor.matmul(
                            a_ps[:, i * C:(i + 1) * C],
                            wkT_s[:, e * C:(e + 1) * C],
                            qbd_s[:],
                        )
                    dh = a16h[:, half * 512:(half + 1) * 512]
                    nc.scalar.copy(dh, a_ps[:])

                # ---- S = Ah.Ch + Ah.Cl + Al.Ch (fp16 x3) ----
                sh0 = spool.tile([C, NJ // 2], f32, tag="sh")
                sh1 = spool.tile([C, NJ // 2], f32, tag="sh")
                s_half = [sh0, sh1]
                for e in range(8):
                    ah = a16h[:, e * C:(e + 1) * C]
                    for half in range(2):
                        s_ps = ps_big.tile([C, 1024], f32, tag="big")
                        # single fp16 term: quantization budget covers it
                        for lhs, cc, st, sp in (
                            (ah, c16h, True, True),
                        ):
                            for n in range(2):
                                col = half * 1024 + n * 512
                                nc.tensor.matmul(
                                    s_ps[:, n * 512:(n + 1) * 512],
                                    lhs, cc[:, col:col + 512],
                                    start=st, stop=sp,
                                )
                        sh = s_half[(e * 2 + half) // 8]
                        off = ((e * 2 + half) % 8) * 1024
                        nc.scalar.copy(sh[:, off:off + 1024], s_ps[:])

                # ---- V^T (fp16 inputs, bf16 out) ----
                vt_s = vpool.tile([C, NJ], bf16, tag="VT")
                for e in range(8):
                    for half in range(2):
                        v_ps = ps_big.tile([C, 1024], f32, tag="big")
                        for n in range(2):
                            col = half * 1024 + n * 512
                            nc.tensor.matmul(
                                v_ps[:, n * 512:(n + 1) * 512],
                                wv_s[:, e * C:(e + 1) * C],
                                c16h[:, col:col + 512],
                            )
                        nc.scalar.copy(
                            vt_s[:, e * NC + half * 1024: e * NC + (half + 1) * 1024],
                            v_ps[:],
                        )

                # ---- per-chunk top8 + local indices (chunk=1024) ----
                cand_s = smpool.tile([C, NCAND], f32, tag="cand")
                li_s = smpool.tile([C, NCAND], dt.uint16, tag="li")
                for ch in range(NCH):
                    sh = s_half[ch // 8]
                    sl = sh[:, (ch % 8) * CHUNK:(ch % 8 + 1) * CHUNK]
                    nc.vector.max(cand_s[:, ch * 8:(ch + 1) * 8], sl)
                for ch in range(NCH):
                    sh = s_half[ch // 8]
                    sl = sh[:, (ch % 8) * CHUNK:(ch % 8 + 1) * CHUNK]
                    nc.vector.max_index(
                        li_s[:, ch * 8:(ch + 1) * 8],
                        cand_s[:, ch * 8:(ch + 1) * 8],
                        sl,
                    )

                # ---- exact top-12 marking on cand ----
                t8a = smpool.tile([C, 8], f32, tag="t8a")
                nc.vector.max(t8a[:], cand_s[:])
                c2 = smpool.tile([C, NCAND], f32, tag="c2")
                nc.vector.match_replace(c2[:], t8a[:], cand_s[:], NEG)
                t8b = smpool.tile([C, 8], f32, tag="t8b")
                nc.vector.max(t8b[:], c2[:])
                nx4 = smpool.tile([C, 8], f32, tag="nx4")
                nc.vector.memset(nx4[:], 1e30)
                nc.vector.tensor_copy(nx4[:, 0:4], t8b[:, 0:4])
                rr = smpool.tile([C, NCAND], f32, tag="rr")
                nc.vector.match_replace(rr[:], nx4[:], c2[:], NEG)
                mask12 = smpool.tile([C, NCAND], f32, tag="mask12")
                nc.vector.tensor_scalar(mask12[:], rr[:], -1e29, None, Alu.is_le)

                # ---- pack global_idx*1024 + q10(value); mask; extract ----
                lif = smpool.tile([C, NCAND], f32, tag="lif")
                nc.vector.tensor_copy(lif[:], li_s[:])
                gfl = smpool.tile([C, NCAND], f32, tag="gfl")
                nc.vector.tensor_scalar(gfl[:], lif[:], 1024.0, None, Alu.mult)
                nc.vector.tensor_add(gfl[:], gfl[:], choff_s[:])
                q10 = smpool.tile([C, NCAND], f32, tag="q10")
                nc.vector.tensor_scalar(
                    q10[:], cand_s[:], 4.0, 128.0, Alu.add, Alu.mult
                )
                nc.vector.tensor_scalar(
                    q10[:], q10[:], 1023.0, 1.0, Alu.min, Alu.max
                )
                pm = smpool.tile([C, NCAND], f32, tag="pm")
                nc.vector.tensor_add(pm[:], gfl[:], q10[:])
                nc.vector.tensor_mul(pm[:], pm[:], mask12[:])

                pw = smpool.tile([C, 16], f32, tag="pw")
                nc.vector.max(pw[:, 0:8], pm[:])
                pm2 = smpool.tile([C, NCAND], f32, tag="pm2")
                nc.vector.match_replace(pm2[:], pw[:, 0:8], pm[:], 0.0)
                nc.vector.max(pw[:, 8:16], pm2[:])

                # ---- decode winners: gidx + value -> weights ----
                gidxf = smpool.tile([C, 16], f32, tag="gidxf")
                nc.vector.tensor_scalar(
                    gidxf[:], pw[:], 1.0 / 1024.0, -0.5, Alu.mult, Alu.add
                )
                nc.vector.tensor_scalar(
                    gidxf[:], gidxf[:], MAGIC, MAGIC, Alu.add, Alu.subtract
                )
                vv = smpool.tile([C, 16], f32, tag="vv")
                nc.vector.tensor_scalar(vv[:], gidxf[:], -1024.0, None, Alu.mult)
                nc.vector.tensor_add(vv[:], vv[:], pw[:])
                nc.vector.tensor_scalar(
                    vv[:], vv[:], 1.0 / 128.0, -4.0, Alu.mult, Alu.add
                )
                expv = smpool.tile([C, 16], f32, tag="expv")
                nc.scalar.activation(
                    expv[:], vv[:], mybir.ActivationFunctionType.Exp
                )
                wmask = smpool.tile([C, 16], f32, tag="wmask")
                nc.vector.tensor_scalar(wmask[:], pw[:], 0.5, None, Alu.is_ge)
                wgt = smpool.tile([C, 16], f32, tag="wgt")
                nc.vector.tensor_mul(wgt[:], expv[:], wmask[:])
                den = smpool.tile([C, 1], f32, tag="den")
                nc.vector.tensor_reduce(
                    den[:], wgt[:], mybir.AxisListType.X, Alu.add
                )
                rden = smpool.tile([C, 1], f32, tag="rden")
                nc.vector.reciprocal(rden[:], den[:])
                wn = smpool.tile([C, 16], f32, tag="wn")
                nc.vector.tensor_scalar(wn[:], wgt[:], rden[:], None, Alu.mult)

                # ---- pair index (bf16 gather needs 4B granules: d=2) ----
                gp = smpool.tile([C, 16], f32, tag="gp")
                nc.vector.tensor_scalar(
                    gp[:], gidxf[:], 0.5, -0.25, Alu.mult, Alu.add
                )
                nc.vector.tensor_scalar(
                    gp[:], gp[:], MAGIC, MAGIC, Alu.add, Alu.subtract
                )
                gp_i = smpool.tile([C, 16], dt.int16, tag="gpi")
                nc.vector.tensor_copy(gp_i[:], gp[:])
                par = smpool.tile([C, 16], f32, tag="par")
                nc.vector.tensor_scalar(par[:], gp[:], -2.0, None, Alu.mult)
                nc.vector.tensor_add(par[:], par[:], gidxf[:])
                parc = smpool.tile([C, 16], f32, tag="parc")
                nc.vector.tensor_scalar(
                    parc[:], par[:], -1.0, 1.0, Alu.mult, Alu.add
                )
                wnE = smpool.tile([C, 16], f32, tag="wnE")
                nc.vector.tensor_mul(wnE[:], wn[:], parc[:])
                wnO = smpool.tile([C, 16], f32, tag="wnO")
                nc.vector.tensor_mul(wnO[:], wn[:], par[:])

                # ---- gather V pairs (per-head core lists) ----
                g_s = smpool.tile([C, 512], bf16, tag="G")
                nc.gpsimd.ap_gather(
                    g_s[:], vt_s[:], gp_i[:],
                    channels=C, num_elems=NJ // 2, d=2, num_idxs=256,
                )

                # ---- weights -> [(h,d), (i,q,parity)] via headrep matmul ----
                wEb = (
                    wnE[:].rearrange("p (i o) -> p i o", o=1)
                    .to_broadcast([C, NQ, 32])
                )
                wOb = (
                    wnO[:].rearrange("p (i o) -> p i o", o=1)
                    .to_broadcast([C, NQ, 32])
                )
                tmpE = smpool.tile([C, 512], f32, tag="tmpE")
                nc.vector.tensor_mul(
                    tmpE[:].rearrange("p (i s) -> p i s", s=32),
                    wEb,
                    me_s[:].rearrange("p (i s) -> p i s", s=32),
                )
                wsc = smpool.tile([C, 512], f32, tag="wsc")
                nc.vector.tensor_mul(
                    wsc[:].rearrange("p (i s) -> p i s", s=32),
                    wOb,
                    mo_s[:].rearrange("p (i s) -> p i s", s=32),
                )
                nc.vector.tensor_add(wsc[:], wsc[:], tmpE[:])
                wb_ps = ps_m.tile([C, 512], f32, tag="misc")
                nc.tensor.matmul(wb_ps[:], hrep_s[:], wsc[:])
                wb_s = smpool.tile([C, 512], bf16, tag="wb")
                nc.scalar.copy(wb_s[:], wb_ps[:])

                gw = smpool.tile([C, 512], f32, tag="gw")
                nc.vector.tensor_mul(gw[:], g_s[:], wb_s[:])
                # reduce over (i, parity), keep q: write PV^T into (q,b) cols
                nc.vector.tensor_reduce(
                    pvt4_s[:, b::BPC],
                    gw[:].rearrange("p (i q r) -> p q i r", q=NQ, r=2),
                    mybir.AxisListType.XY,
                    Alu.add,
                )

            # ---- final projections for all 4 batches ----
            o1_ps = ps_m.tile([C, 512], f32, tag="misc")
            for q in range(NQ):
                nc.tensor.matmul(
                    o1_ps[:, 0:BPC],
                    wjwp_s[:, q * C:(q + 1) * C],
                    pvt4_s[:, q * BPC:(q + 1) * BPC],
                    start=(q == 0),
                    stop=(q == NQ - 1),
                )
            o2_s = smpool.tile([C, BPC], f32, tag="o2")
            nc.vector.tensor_add(o2_s[:], o1_ps[:, 0:BPC], xT_s[:])
            o3_ps = ps_m.tile([C, 512], f32, tag="misc")
            nc.tensor.matmul(o3_ps[0:BPC, 0:C], o2_s[:], wp_s[:])
            o4_s = smpool.tile([BPC, C], f32, tag="o4")
            nc.vector.tensor_add(o4_s[:], o3_ps[0:BPC, 0:C], bp4_s[:])
            nc.sync.dma_start(out_d[:], o4_s[:])

    nc.compile()
    return nc


def _host_prep(inputs):
    x = np.asarray(inputs["x"], dtype=np.float32)              # [32, 1, 128]
    complement = np.asarray(inputs["complement"], np.float32)  # [32, 2047, 128]
    Wq = np.asarray(inputs["Wq"], np.float32)
    Wkv = np.asarray(inputs["Wkv"], np.float32)
    Wjw = np.asarray(inputs["Wjw"], np.float32)
    Wp = np.asarray(inputs["Wp"], np.float32)
    bp = np.asarray(inputs["bp"], np.float32)

    wkT = np.empty((C, 8 * C), np.float32)
    wv = np.empty((C, 8 * C), np.float32)
    for e in range(8):
        wkT[:, e * C:(e + 1) * C] = Wkv[:, e * 256: e * 256 + 128].T
        wv[:, e * C:(e + 1) * C] = Wkv[:, e * 256 + 128: e * 256 + 256]
    wv = wv.astype(np.float16)
    # Wjw rows are (h,q,d); per-q slice with rows (h,d)
    wjwp = (
        Wjw.reshape(H, NQ, HD, C).transpose(1, 0, 2, 3).reshape(NQ, C, C)
        .transpose(1, 0, 2).reshape(C, NQ * C)
    )
    bp4 = np.tile(bp.reshape(1, C), (BPC, 1)).astype(np.float32)
    hrep = np.kron(np.eye(H, dtype=np.float32), np.ones((HD, HD), np.float32))
    choffrow = ((np.arange(NCAND) // 8) * (CHUNK * 1024)).astype(np.float32)
    choff = np.tile(choffrow.reshape(1, NCAND), (C, 1))
    s_idx = np.tile(np.arange(32).reshape(1, 1, 32), (C, NQ, 1))
    p_idx = (np.arange(C) % NQ).reshape(C, 1, 1)
    me = (s_idx == 2 * p_idx).astype(np.float32).reshape(C, 512)
    mo = (s_idx == 2 * p_idx + 1).astype(np.float32).reshape(C, 512)

    shared = dict(
        wq=np.ascontiguousarray(Wq),
        wkT=np.ascontiguousarray(wkT),
        wv=np.ascontiguousarray(wv),
        wjwp=np.ascontiguousarray(wjwp),
        wp=np.ascontiguousarray(Wp),
        bp4=bp4,
        hrep=np.ascontiguousarray(hrep),
        choff=np.ascontiguousarray(choff),
        me=np.ascontiguousarray(me),
        mo=np.ascontiguousarray(mo),
    )

    in_maps = []
    for core in range(CORES):
        bs = range(core * BPC, (core + 1) * BPC)
        comp = np.stack(
            [
                np.concatenate([x[b].reshape(1, C), complement[b]], axis=0)
                for b in bs
            ]
        ).astype(np.float32)
        compT = comp.transpose(0, 2, 1)          # [BPC, C, NC]
        comphT = compT.astype(np.float16)
        complT = (compT - comphT.astype(np.float32)).astype(np.float16)
        xT = np.ascontiguousarray(x[list(bs)].reshape(BPC, C).T)
        m = dict(shared)
        m["comphT"] = np.ascontiguousarray(comphT)
        m["complT"] = np.ascontiguousarray(complT)
        m["xT"] = xT
        in_maps.append(m)
    return in_maps


def kernel(**inputs):
    from concourse.bass_utils import run_bass_kernel_spmd

    if "prog" not in _prog_cache:
        _prog_cache["prog"] = _build_program()
    nc = _prog_cache["prog"]

    in_maps = _host_prep(inputs)
    res = run_bass_kernel_spmd(nc, in_maps, core_ids=list(range(CORES)))
    out = np.empty((B, 1, C), np.float32)
    for core in range(CORES):
        o = res.results[core]["out"]
        for i in range(BPC):
            out[core * BPC + i, 0, :] = o[i]
    return out


if __name__ == "__main__":
    d = np.load("/root/problem/inputs_cache.npz")
    inputs = {k: d[k] for k in d.files}
    got = kernel(**inputs)
    print("kernel output:", got.shape, got.dtype, np.abs(got).max())

